# revision 52
# baseline (speedup 1.0000x reference)
"""Self-contained Trainium2 Bass kernel for nn_Attention_87282325389426.

GQA attention with "raw reshape" (scrambled) head semantics:
  B=2, S=2048, D=2048, HQ=16, HK=HV=4, DQK=128, DV=512.

Sharding: the raw-reshape semantics decompose exactly into B*HK = 8
independent (batch, kv-group) units -> one per NeuronCore, zero collectives.
Core i handles (b, k) = (i//4, i%4):
  - queries  : x rows in spans {(4g+k)*128 : g=0..3}   (512 rows)
  - keys/vals: x rows [512k, 512(k+1))                 (512 rows)
  - output   : full 2048-wide rows for the 4 query spans (disjoint across cores)

Per-core pipeline (layouts chosen so NO on-chip transposes are needed):
  QT[d, (g,ds,h)]  = WQ_hblock^T @ xT_q      (projection emits transposed Q)
  KT[d, (h,ds)]    = WK_hblock^T @ xT_kv
  V  [s, (h,dv)]   natural
  ST[t'=(h,ds'), t=(ds,h')] = KT_tile^T @ QT_block  (scores transposed, bf16)
  P = exp(ST/sqrt(128)) * causal_mask  (mask only on diagonal tiles; upper
      blocks skipped entirely -> ~37% of score/PV work elided)
  OT[dv, t] += V_chunk^T @ P_chunk   (PV emits transposed O directly; row
      sums replicated across partitions via ones[128,128] matmul on the PE)
  y[s, n] = (OT/rowsum via strided APs) @ W0 + b0    (W0 in bf16)

Scheduling notes (perf):
  - x / bias loads issue from the gpsimd DMA queue so they never head-of-line
    block the weight stream on the sync queue.
  - W0 tiles are all issued on the sync queue right after the projection
    weights; the ring guards self-pace them through the attention phase
    (DMA is otherwise idle there), so the output GEMM never waits on HBM.
  - Attention runs 4 single-PSUM-bank PV passes per block; next block's
    score matmuls are interleaved into passes 1-3 so the PE never has a
    pure-scores stretch (which would stall on the scalar-engine exp rate)
    and never idles long enough for the HAM clock-gate to re-throttle.
  - PSUM normalization drains are split between vector and gpsimd.
"""

import math
import os

# Whole-tile dependency granularity: the kernel's strided multi-dim write APs
# (qt / ot drains) must not rely on subtile overlap tracking.
os.environ["BY_DEFAULT_DISABLE_SUBTILE_DEPS"] = "1"

import numpy as np

import concourse.bass as bass  # noqa: F401
import concourse.mybir as mybir
import concourse.tile as tile
from concourse import bacc
from concourse.bass_utils import run_bass_kernel_spmd

F32 = mybir.dt.float32
BF16 = mybir.dt.bfloat16
EXP = mybir.ActivationFunctionType.Exp
IDENT = mybir.ActivationFunctionType.Identity
GE = mybir.AluOpType.is_ge

D = 2048
SCALE = 1.0 / math.sqrt(128.0)


def _body(tc, ctx, xtq, xtkv, wq, wk, wv, w0, bq, bk, bv, b0, out):
    nc = tc.nc

    pers = ctx.enter_context(tc.tile_pool(name="pers", bufs=1))
    # [d, g, qb, m] with block columns m = h*32 + ds (h-outer): each score
    # block's moving operand is one contiguous 512-col slice, and the
    # projection-psum drains write 32-element runs instead of stride-16.
    qt = pers.tile([128, 4, 4, 512], BF16)
    kt = pers.tile([128, 4 * 512], BF16)      # free = h*512 + ds
    # per-s-tile V tiles [s-part, (h,dv)] so early attention blocks only
    # depend on the s-tiles they read
    vsb_t = [pers.tile([128, 2048], BF16, name=f"vsb_{st}") for st in range(4)]
    ones = pers.tile([128, 128], BF16)
    maskt = pers.tile([128, 4, 512], BF16)    # per-h diagonal causal mask
    mask1 = pers.tile([128, 512], BF16)
    bq_sb = pers.tile([128, 16], F32)
    bk_sb = pers.tile([128, 4], F32)
    bv_sb = pers.tile([1, D], F32)
    b0_sb = pers.tile([1, D], F32)
    b0bf = pers.tile([1, D], BF16)
    bvbf = pers.tile([1, D], BF16)
    ones_row = pers.tile([1, 128], BF16)

    # ---------------- projections (all bf16 operands) ----------------
    # score-tile pool opened early: blocks (0,0) and (0,1) get their score
    # matmuls pre-emitted into the V-projection passes.
    ppool = ctx.enter_context(tc.tile_pool(name="ppool", bufs=32))

    qt_h = qt.rearrange("p g qb (h ds) -> p (g qb) h ds", h=16)

    def make_emitters(g, qb, ps_tile):
        """Score-tile emitters for block (g, qb): each emits one score
        matmul + exp (+ diagonal mask on gpsimd), appending to pts.

        Score columns are iterated h-outer (m = h*32 + ds, not s' = ds*16+h)
        so the moving operand AP is near-contiguous; the causal mask and the
        output-GEMM stationary AP use the same permuted order."""
        rhs_q = qt[:, g, qb, :]
        pts = []

        def mk(m2, h):
            def em():
                ps = ps_tile(g, qb, m2, h)
                nc.tensor.matmul(
                    ps, kt[:, h * 512 + m2 * 128: h * 512 + (m2 + 1) * 128],
                    rhs_q, start=True, stop=True)
                pt = ppool.tile([128, 512], BF16, tag="pt",
                                name=f"pt_{g}_{qb}_{m2}_{h}")
                nc.scalar.activation(pt, ps, EXP, scale=SCALE)
                if m2 == qb:
                    nc.gpsimd.tensor_mul(pt, pt, maskt[:, h, :])
                pts.append((m2, h, pt))
            return em

        ems = [mk(m2, h) for m2 in range(qb + 1) for h in range(4)]
        return pts, ems

    with (
        tc.tile_pool(name="xp", bufs=1) as xp,
        tc.tile_pool(name="ws", bufs=8) as ws,
        tc.tile_pool(name="pps", bufs=8, space="PSUM") as pps,
    ):
        # x tiles split per DMA so whole-tile dependencies don't serialize
        # the first matmuls behind the full x load.
        xq_chunks = [(0, 2), (2, 4), (6, 5), (11, 5)]
        xkv_chunks = [(0, 4), (4, 4), (8, 4), (12, 4)]
        xtq_t = [xp.tile([128, w, 512], BF16, name=f"xtq_{a}") for a, w in xq_chunks]
        xtkv_t = [xp.tile([128, w, 512], BF16, name=f"xtkv_{a}")
                  for a, w in xkv_chunks]

        def x_chunk(tiles, chunks, cc):
            for (a, w), t in zip(chunks, tiles):
                if a <= cc < a + w:
                    return t[:, cc - a, :]
            raise AssertionError(cc)

        xtq_r = xtq[:, :].rearrange("(cc p) n -> p cc n", p=128)
        xtkv_r = xtkv[:, :].rearrange("(cc p) n -> p cc n", p=128)

        # x / bias loads on the scalar issue queue (a *hardware* dynamic DMA
        # queue, unlike gpsimd's software queue) so they never head-of-line
        # block the weight stream on the sync queue.  First chunk first so
        # the PE can start as soon as the first weight tile lands.
        for (a, w), t in zip(xq_chunks, xtq_t):
            nc.scalar.dma_start(out=t, in_=xtq_r[:, a:a + w, :])
        for (a, w), t in zip(xkv_chunks, xtkv_t):
            nc.scalar.dma_start(out=t, in_=xtkv_r[:, a:a + w, :])
        nc.scalar.dma_start(out=bq_sb, in_=bq[:, :])
        nc.scalar.dma_start(out=bk_sb, in_=bk[:, :])
        nc.scalar.dma_start(out=bv_sb, in_=bv[0:1, :])
        nc.scalar.dma_start(out=b0_sb, in_=b0[0:1, :])

        nc.vector.memset(ones, 1.0)
        nc.vector.memset(mask1, 1.0)
        nc.vector.memset(ones_row, 1.0)
        # mask[p, hk, m=hq*32+ds] = 1.0 iff query s'=ds*16+hq >= key 4p+hk,
        # i.e. 16*ds - 4p + (hq - hk) >= 0, in the permuted column order.
        for hk in range(4):
            for hq in range(16):
                nc.gpsimd.affine_select(
                    out=maskt[:, hk, 32 * hq:32 * (hq + 1)], in_=mask1[:, 0:32],
                    pattern=[[16, 32]], compare_op=GE, fill=0.0,
                    base=hq - hk, channel_multiplier=-4,
                )
        nc.vector.tensor_copy(b0bf, b0_sb)
        nc.vector.tensor_copy(bvbf, bv_sb)

        # Q: four quarter-column passes over WQ, 4 psum banks each -- with
        # the 8-slot psum ring, each pass's banks were drained a full pass
        # ago, so pass boundaries never stall.  First two weight tiles
        # single-cc (fast start), then 2-cc batched.
        wq_chunks = [(0, 1), (1, 1)] + [(2 + 2 * i, 2) for i in range(7)]
        wq_c = wq[:, :].rearrange("(cc p) n -> p cc n", p=128)
        for qp in range(4):
            pq = [pps.tile([128, 512], F32, tag="pj", name=f"pq{qp}_{i}")
                  for i in range(4)]
            for a, w in wq_chunks:
                wt = ws.tile([128, 2, 512], BF16, tag="wq", bufs=12)
                nc.sync.dma_start(out=wt[:, 0:w, :],
                                  in_=wq_c[:, a:a + w, qp * 512:(qp + 1) * 512])
                for sub in range(w):
                    cc = a + sub
                    for ci in range(4):
                        nc.tensor.matmul(pq[ci], wt[:, sub, ci * 128:(ci + 1) * 128],
                                         x_chunk(xtq_t, xq_chunks, cc),
                                         start=(cc == 0), stop=(cc == 15))
            # drains split across vector and scalar so the next psum pass
            # isn't gated on a single engine's drain rate
            for ci in range(4):
                h = qp * 4 + ci
                if ci % 2 == 0:
                    nc.vector.tensor_scalar_add(
                        qt_h[:, :, h, :],
                        pq[ci].rearrange("p (gq ds) -> p gq ds", gq=16),
                        bq_sb[:, h:h + 1])
                else:
                    nc.scalar.activation(
                        qt_h[:, :, h, :],
                        pq[ci].rearrange("p (gq ds) -> p gq ds", gq=16),
                        IDENT, bias=bq_sb[:, h:h + 1])

        # K: one pass, 4 banks, 4-cc batched weight DMAs
        wk_c = wk[:, :].rearrange("(cc p) n -> p cc n", p=128)
        pk = [pps.tile([128, 512], F32, tag="pj", name=f"pk_{i}") for i in range(4)]
        for ccp in range(4):
            wt = ws.tile([128, 4, 512], BF16, tag="wk", bufs=4)
            nc.sync.dma_start(out=wt, in_=wk_c[:, 4 * ccp:4 * ccp + 4, :])
            for sub in range(4):
                cc = 4 * ccp + sub
                for h in range(4):
                    nc.tensor.matmul(pk[h], wt[:, sub, h * 128:(h + 1) * 128],
                                     x_chunk(xtkv_t, xkv_chunks, cc),
                                     start=(cc == 0), stop=(cc == 15))
        for h in range(4):
            if h % 2 == 0:
                nc.vector.tensor_scalar_add(kt[:, h * 512:(h + 1) * 512], pk[h],
                                            bk_sb[:, h:h + 1])
            else:
                nc.scalar.activation(kt[:, h * 512:(h + 1) * 512], pk[h],
                                     IDENT, bias=bk_sb[:, h:h + 1])

        # first two attention blocks' score pipelines, pre-emitted into the
        # V-projection passes below (they only need qt/kt, not vsb)
        def pj_ps(g, qb, m2, h):
            return pps.tile([128, 512], F32, tag="pj", name=f"st_{g}_{qb}_{m2}_{h}")

        pts0, ems0 = make_emitters(0, 0, pj_ps)
        pts1, ems1 = make_emitters(0, 1, pj_ps)
        vpre = {1: ems0, 2: ems1[0:4], 3: ems1[4:8]}

        # V natural [s, (h,dv)]; four (st-pair, nb-pair) passes, 4 psum
        # banks each.  Bias row seeds the accumulators (K=1 matmul), so the
        # drains are plain copies and can split across vector/scalar.
        wv_c = wv[:, :].rearrange("(cc p) n -> p cc n", p=128)
        for stp in range(2):
            for nbp in range(2):
                vp = stp * 2 + nbp
                ems = vpre.get(vp, [])
                psv = [pps.tile([128, 512], F32, tag="pj", name=f"psv{stp}{nbp}_{i}")
                       for i in range(4)]
                for i in range(4):
                    nb = nbp * 2 + i % 2
                    nc.tensor.matmul(psv[i], ones_row,
                                     bvbf[0:1, nb * 512:(nb + 1) * 512],
                                     start=True, stop=False)
                for ccp in range(8):
                    wt = ws.tile([128, 2, 1024], BF16, tag="wv", bufs=6)
                    nc.sync.dma_start(
                        out=wt,
                        in_=wv_c[:, 2 * ccp:2 * ccp + 2, nbp * 1024:(nbp + 1) * 1024])
                    for sub in range(2):
                        cc = 2 * ccp + sub
                        for sti in range(2):
                            st = stp * 2 + sti
                            for nbi in range(2):
                                nc.tensor.matmul(
                                    psv[sti * 2 + nbi],
                                    x_chunk(xtkv_t, xkv_chunks, cc)[
                                        :, st * 128:(st + 1) * 128],
                                    wt[:, sub, nbi * 512:(nbi + 1) * 512],
                                    start=False, stop=(cc == 15))
                    if ems and ccp % 2 == 1:
                        ems[ccp // 2]()
                for sti in range(2):
                    st = stp * 2 + sti
                    for nbi in range(2):
                        nb = nbp * 2 + nbi
                        if nbi == 0:
                            nc.vector.tensor_copy(
                                vsb_t[st][:, nb * 512:(nb + 1) * 512],
                                psv[sti * 2 + nbi])
                        else:
                            nc.scalar.activation(
                                vsb_t[st][:, nb * 512:(nb + 1) * 512],
                                psv[sti * 2 + nbi],
                                mybir.ActivationFunctionType.Copy)

    # W0 stream: issue everything now on the sync queue; the ring guards
    # (bufs=12) park the queue and self-pace the prefetch through the
    # attention phase, where DMA is otherwise idle.
    w0_r = w0[:, :].rearrange("(cj p) n -> p cj n", p=128)   # cj = j*4 + dvc
    w0pool = ctx.enter_context(tc.tile_pool(name="w0s", bufs=1))
    w0tiles = []
    for nb in range(4):
        for cjp in range(32):
            wt0 = w0pool.tile([128, 2, 512], BF16, tag="w0", bufs=16,
                              name=f"w0_{nb}_{cjp}")
            nc.sync.dma_start(
                out=wt0, in_=w0_r[:, 2 * cjp:2 * cjp + 2, nb * 512:(nb + 1) * 512])
            w0tiles.append(wt0)

    # ---------------- attention ----------------
    otpool = ctx.enter_context(tc.tile_pool(name="otpool", bufs=1))
    # [dv-part, g*4+dvc, t] with t ordered (h, qb, ds): the output GEMM's
    # stationary slice for head j is then one contiguous 128-col run.
    ot = otpool.tile([128, 16, 2048], BF16)
    blocks = [(g, qb) for g in range(4) for qb in range(4)]
    with (
        tc.tile_pool(name="rp", bufs=4) as rp,
        tc.tile_pool(name="stps", bufs=3, space="PSUM") as stps,
        tc.tile_pool(name="otps", bufs=4, space="PSUM") as otps,
        tc.tile_pool(name="sumps", bufs=1, space="PSUM") as sumps,
    ):
        def st_ps(g, qb, m2, h):
            return stps.tile([128, 512], F32, tag="st",
                             name=f"st_{g}_{qb}_{m2}_{h}")

        pre_pts = {0: pts0, 1: pts1}
        cur_pts = pts0
        for bi, (g, qb) in enumerate(blocks):
            n = 4 * (qb + 1)
            assert len(cur_pts) == n
            if bi + 1 >= len(blocks):
                next_pts, next_ems = [], []
            elif bi + 1 in pre_pts:
                next_pts, next_ems = pre_pts[bi + 1], []
            else:
                next_pts, next_ems = make_emitters(*blocks[bi + 1], st_ps)
            otp = [otps.tile([128, 512], F32, tag="otp", name=f"otp{p}_{g}_{qb}")
                   for p in range(4)]
            smp = sumps.tile([128, 512], F32, tag="smp", name=f"smp_{g}_{qb}")
            # pass 0: dvc0 + rowsum interleaved
            for i, (m2, h, pt) in enumerate(cur_pts):
                first, last = (i == 0), (i == n - 1)
                nc.tensor.matmul(otp[0],
                                 vsb_t[m2][:, h * 512: h * 512 + 128],
                                 pt, start=first, stop=last)
                nc.tensor.matmul(smp, ones, pt, start=first, stop=last)
            # reciprocal as exp(-ln(x)) on the scalar engine: off the vector
            # critical path, and plenty accurate for softmax denominators
            rln = rp.tile([128, 512], F32, tag="rln", name=f"rln_{g}_{qb}",
                          bufs=2)
            rcb = rp.tile([128, 512], F32, tag="rcb", name=f"rcb_{g}_{qb}")
            nc.scalar.activation(rln, smp, mybir.ActivationFunctionType.Ln)
            nc.scalar.activation(rcb, rln, EXP, scale=-1.0)
            # passes 1-3, with the next block's score pipeline interleaved
            total_slots = 3 * n
            slot = 0
            done = 0
            for p in range(1, 4):
                for i, (m2, h, pt) in enumerate(cur_pts):
                    first, last = (i == 0), (i == n - 1)
                    nc.tensor.matmul(
                        otp[p],
                        vsb_t[m2][:, h * 512 + p * 128: h * 512 + (p + 1) * 128],
                        pt, start=first, stop=last)
                    slot += 1
                    # front-loaded: all of the next block's scores are
                    # emitted by mid-pass-2, so exp+mask latency is hidden
                    want = min(len(next_ems),
                               (2 * slot * len(next_ems)) // total_slots)
                    while done < want:
                        next_ems[done]()
                        done += 1
                # drain the pass finished one iteration ago
                nc.vector.tensor_mul(
                    ot[:, 4 * g + p - 1, :].rearrange(
                        "p (h qb ds) -> p h qb ds", h=16, qb=4)[:, :, qb, :],
                    otp[p - 1].rearrange("p (h ds) -> p h ds", h=16),
                    rcb.rearrange("p (h ds) -> p h ds", h=16))
            while done < len(next_ems):
                next_ems[done]()
                done += 1
            nc.vector.tensor_mul(
                ot[:, 4 * g + 3, :].rearrange(
                    "p (h qb ds) -> p h qb ds", h=16, qb=4)[:, :, qb, :],
                otp[3].rearrange("p (h ds) -> p h ds", h=16),
                rcb.rearrange("p (h ds) -> p h ds", h=16))
            cur_pts = next_pts

    # ---------------- output GEMM ----------------
    with (
        tc.tile_pool(name="yp", bufs=4) as yp,
        tc.tile_pool(name="yps", bufs=8, space="PSUM") as yps,
    ):
        ti = 0
        for nb in range(4):
            ypsum = [yps.tile([128, 512], F32, tag="y", name=f"yps{nb}_{g}")
                     for g in range(4)]
            # bias row: y += ones_col^T(K=1) @ b0_row
            for g in range(4):
                nc.tensor.matmul(
                    ypsum[g], ones_row, b0bf[0:1, nb * 512:(nb + 1) * 512],
                    start=True, stop=False)
            for cjp in range(32):
                wt0 = w0tiles[ti]
                ti += 1
                for sub in range(2):
                    cj = 2 * cjp + sub
                    j, dvc = cj // 4, cj % 4
                    for g in range(4):
                        lt = ot[:, 4 * g + dvc, 128 * j:128 * (j + 1)]
                        nc.tensor.matmul(ypsum[g], lt, wt0[:, sub, :],
                                         start=False, stop=(cj == 63))
            for g in range(4):
                yt = yp.tile([128, 512], BF16, tag="y", name=f"yt{nb}_{g}")
                nc.vector.tensor_copy(yt, ypsum[g])
                nc.scalar.dma_start(
                    out=out[g * 128:(g + 1) * 128, nb * 512:(nb + 1) * 512],
                    in_=yt)


def build_graph():
    nc = bacc.Bacc(None, target_bir_lowering=False)
    xtq = nc.declare_dram_parameter("xtq", [D, 512], BF16, isOutput=False)
    xtkv = nc.declare_dram_parameter("xtkv", [D, 512], BF16, isOutput=False)
    wq = nc.declare_dram_parameter("wq", [D, D], BF16, isOutput=False)
    wk = nc.declare_dram_parameter("wk", [D, 512], BF16, isOutput=False)
    wv = nc.declare_dram_parameter("wv", [D, D], BF16, isOutput=False)
    w0 = nc.declare_dram_parameter("w0", [8192, D], BF16, isOutput=False)
    bq = nc.declare_dram_parameter("bq", [128, 16], F32, isOutput=False)
    bk = nc.declare_dram_parameter("bk", [128, 4], F32, isOutput=False)
    bv = nc.declare_dram_parameter("bv", [1, D], F32, isOutput=False)
    b0 = nc.declare_dram_parameter("b0", [1, D], F32, isOutput=False)
    out = nc.declare_dram_parameter("out", [512, D], BF16, isOutput=True)
    from contextlib import ExitStack
    with tile.TileContext(nc) as tc, ExitStack() as ctx:
        _body(tc, ctx, xtq, xtkv, wq, wk, wv, w0, bq, bk, bv, b0, out)
    nc.finalize()
    return nc


_CACHE = {}


def _get_nc():
    if "nc" not in _CACHE:
        _CACHE["nc"] = build_graph()
    return _CACHE["nc"]


def _prep_in_maps(x, WQ, bQ, WK, bK, WV, bV, W0, b0):
    bf16 = mybir.dt.np(BF16)
    x = np.asarray(x, np.float32)
    w0_bf = np.ascontiguousarray(np.asarray(W0, np.float32).astype(bf16))
    wq_bf = np.ascontiguousarray(np.asarray(WQ, np.float32).astype(bf16))
    wk_bf = np.ascontiguousarray(np.asarray(WK, np.float32).astype(bf16))
    wv_bf = np.ascontiguousarray(np.asarray(WV, np.float32).astype(bf16))
    bq_r = np.ascontiguousarray(np.asarray(bQ, np.float32).reshape(16, 128).T)
    bk_r = np.ascontiguousarray(np.asarray(bK, np.float32).reshape(4, 128).T)
    bv_r = np.ascontiguousarray(np.asarray(bV, np.float32).reshape(1, D))
    b0_r = np.ascontiguousarray(np.asarray(b0, np.float32).reshape(1, D))
    in_maps = []
    for core in range(8):
        b, k = core // 4, core % 4
        q_rows = np.concatenate(
            [np.arange((4 * g + k) * 128, (4 * g + k + 1) * 128) for g in range(4)])
        xtq = np.ascontiguousarray(x[b, q_rows, :].T.astype(bf16))
        xtkv = np.ascontiguousarray(x[b, 512 * k:512 * (k + 1), :].T.astype(bf16))
        in_maps.append({
            "xtq": xtq, "xtkv": xtkv, "wq": wq_bf, "wk": wk_bf, "wv": wv_bf,
            "w0": w0_bf, "bq": bq_r, "bk": bk_r, "bv": bv_r, "b0": b0_r,
        })
    return in_maps


def _install_ntff_hook_shim():
    """The image's antenv lacks axon_hooks; provide it so trace=True works."""
    import sys
    import types
    if "antenv.axon_hooks" in sys.modules:
        return
    mod = types.ModuleType("antenv.axon_hooks")
    mod._hook = None

    def set_axon_ntff_profile_hook(h):
        mod._hook = h

    def get_axon_ntff_profile_hook():
        return mod._hook

    mod.set_axon_ntff_profile_hook = set_axon_ntff_profile_hook
    mod.get_axon_ntff_profile_hook = get_axon_ntff_profile_hook
    sys.modules["antenv.axon_hooks"] = mod
    try:
        from trn_agent_boot.trn_boot import _ntff_profile_via_ctypes
        mod._hook = _ntff_profile_via_ctypes("/opt/axon/libaxon_pjrt.so")
    except Exception as e:  # pragma: no cover
        print("ntff shim: hook unavailable:", e)


def run(inputs, trace=False, tmpdir=None, return_res=False):
    """Run on 8 cores; returns (full_output, exec_time_ns_or_None)."""
    if trace:
        _install_ntff_hook_shim()
    in_maps = _prep_in_maps(
        inputs["x"], inputs["WQ"], inputs["bQ"], inputs["WK"], inputs["bK"],
        inputs["WV"], inputs["bV"], inputs["W0"], inputs["b0"])
    res = run_bass_kernel_spmd(_get_nc(), in_maps, core_ids=list(range(8)), trace=trace,
                               tmpdir=tmpdir)
    full = np.zeros((2, 2048, 2048), np.float32)
    for core in range(8):
        b, k = core // 4, core % 4
        co = np.asarray(res.results[core]["out"], dtype=np.float32)
        for g in range(4):
            full[b, (4 * g + k) * 128:(4 * g + k + 1) * 128, :] = co[g * 128:(g + 1) * 128, :]
    if return_res:
        return full, res
    return full, res.exec_time_ns


def kernel(**inputs):
    out, _ = run(inputs, trace=False)
    return out


# revision 53
# speedup vs baseline: 1.0329x; 1.0329x over previous
"""Self-contained Trainium2 Bass kernel for nn_Attention_87282325389426.

GQA attention with "raw reshape" (scrambled) head semantics:
  B=2, S=2048, D=2048, HQ=16, HK=HV=4, DQK=128, DV=512.

Sharding: the raw-reshape semantics decompose exactly into B*HK = 8
independent (batch, kv-group) units -> one per NeuronCore, zero collectives.
Core i handles (b, k) = (i//4, i%4):
  - queries  : x rows in spans {(4g+k)*128 : g=0..3}   (512 rows)
  - keys/vals: x rows [512k, 512(k+1))                 (512 rows)
  - output   : full 2048-wide rows for the 4 query spans (disjoint across cores)

Per-core pipeline (layouts chosen so NO on-chip transposes are needed):
  QT[d, (g,ds,h)]  = WQ_hblock^T @ xT_q      (projection emits transposed Q)
  KT[d, (h,ds)]    = WK_hblock^T @ xT_kv
  V  [s, (h,dv)]   natural
  ST[t'=(h,ds'), t=(ds,h')] = KT_tile^T @ QT_block  (scores transposed, bf16)
  P = exp(ST/sqrt(128)) * causal_mask  (mask only on diagonal tiles; upper
      blocks skipped entirely -> ~37% of score/PV work elided)
  OT[dv, t] += V_chunk^T @ P_chunk   (PV emits transposed O directly; row
      sums replicated across partitions via ones[128,128] matmul on the PE)
  y[s, n] = (OT/rowsum via strided APs) @ W0 + b0    (W0 in bf16)

Scheduling notes (perf):
  - x / bias loads issue from the gpsimd DMA queue so they never head-of-line
    block the weight stream on the sync queue.
  - W0 tiles are all issued on the sync queue right after the projection
    weights; the ring guards self-pace them through the attention phase
    (DMA is otherwise idle there), so the output GEMM never waits on HBM.
  - Attention runs 4 single-PSUM-bank PV passes per block; next block's
    score matmuls are interleaved into passes 1-3 so the PE never has a
    pure-scores stretch (which would stall on the scalar-engine exp rate)
    and never idles long enough for the HAM clock-gate to re-throttle.
  - PSUM normalization drains are split between vector and gpsimd.
"""

import math
import os

# Whole-tile dependency granularity: the kernel's strided multi-dim write APs
# (qt / ot drains) must not rely on subtile overlap tracking.
os.environ["BY_DEFAULT_DISABLE_SUBTILE_DEPS"] = "1"

import numpy as np

import concourse.bass as bass  # noqa: F401
import concourse.mybir as mybir
import concourse.tile as tile
from concourse import bacc
from concourse.bass_utils import run_bass_kernel_spmd

F32 = mybir.dt.float32
BF16 = mybir.dt.bfloat16
EXP = mybir.ActivationFunctionType.Exp
IDENT = mybir.ActivationFunctionType.Identity
GE = mybir.AluOpType.is_ge

D = 2048
SCALE = 1.0 / math.sqrt(128.0)


def _body(tc, ctx, xtq, xtkv, wq, wk, wv, w0, bq, bk, bv, b0, out):
    nc = tc.nc

    pers = ctx.enter_context(tc.tile_pool(name="pers", bufs=1))
    # [d, g, qb, m] with block columns m = h*32 + ds (h-outer): each score
    # block's moving operand is one contiguous 512-col slice, and the
    # projection-psum drains write 32-element runs instead of stride-16.
    qt = pers.tile([128, 4, 4, 512], BF16)
    kt = pers.tile([128, 4 * 512], BF16)      # free = h*512 + ds
    # per-s-tile V tiles [s-part, (h,dv)] so early attention blocks only
    # depend on the s-tiles they read
    vsb_t = [pers.tile([128, 2048], BF16, name=f"vsb_{st}") for st in range(4)]
    ones = pers.tile([128, 128], BF16)
    maskt = pers.tile([128, 4, 512], BF16)    # per-h diagonal causal mask
    mask1 = pers.tile([128, 512], BF16)
    bq_sb = pers.tile([128, 16], F32)
    bk_sb = pers.tile([128, 4], F32)
    bv_sb = pers.tile([1, D], F32)
    b0_sb = pers.tile([1, D], F32)
    b0bf = pers.tile([1, D], BF16)
    bvbf = pers.tile([1, D], BF16)
    ones_row = pers.tile([1, 128], BF16)

    # ---------------- projections (all bf16 operands) ----------------
    # score-tile pool opened early: blocks (0,0) and (0,1) get their score
    # matmuls pre-emitted into the V-projection passes.
    ppool = ctx.enter_context(tc.tile_pool(name="ppool", bufs=32))

    qt_h = qt.rearrange("p g qb (h ds) -> p (g qb) h ds", h=16)

    def make_emitters(g, qb, ps_tile):
        """Score-tile emitters for block (g, qb): each emits one score
        matmul + exp (+ diagonal mask on gpsimd), appending to pts.

        Score columns are iterated h-outer (m = h*32 + ds, not s' = ds*16+h)
        so the moving operand AP is near-contiguous; the causal mask and the
        output-GEMM stationary AP use the same permuted order."""
        rhs_q = qt[:, g, qb, :]
        pts = []

        def mk(m2, h):
            def em():
                ps = ps_tile(g, qb, m2, h)
                nc.tensor.matmul(
                    ps, kt[:, h * 512 + m2 * 128: h * 512 + (m2 + 1) * 128],
                    rhs_q, start=True, stop=True)
                pt = ppool.tile([128, 512], BF16, tag="pt",
                                name=f"pt_{g}_{qb}_{m2}_{h}")
                nc.scalar.activation(pt, ps, EXP, scale=SCALE)
                if m2 == qb:
                    nc.gpsimd.tensor_mul(pt, pt, maskt[:, h, :])
                pts.append((m2, h, pt))
            return em

        ems = [mk(m2, h) for m2 in range(qb + 1) for h in range(4)]
        return pts, ems

    with (
        tc.tile_pool(name="xp", bufs=1) as xp,
        tc.tile_pool(name="ws", bufs=8) as ws,
        tc.tile_pool(name="pps", bufs=8, space="PSUM") as pps,
    ):
        # x tiles split per DMA so whole-tile dependencies don't serialize
        # the first matmuls behind the full x load.
        xq_chunks = [(0, 2), (2, 4), (6, 5), (11, 5)]
        xkv_chunks = [(0, 4), (4, 4), (8, 4), (12, 4)]
        xtq_t = [xp.tile([128, w, 512], BF16, name=f"xtq_{a}") for a, w in xq_chunks]
        xtkv_t = [xp.tile([128, w, 512], BF16, name=f"xtkv_{a}")
                  for a, w in xkv_chunks]

        def x_chunk(tiles, chunks, cc):
            for (a, w), t in zip(chunks, tiles):
                if a <= cc < a + w:
                    return t[:, cc - a, :]
            raise AssertionError(cc)

        xtq_r = xtq[:, :].rearrange("(cc p) n -> p cc n", p=128)
        xtkv_r = xtkv[:, :].rearrange("(cc p) n -> p cc n", p=128)

        # x / bias loads on the scalar issue queue (a *hardware* dynamic DMA
        # queue, unlike gpsimd's software queue) so they never head-of-line
        # block the weight stream on the sync queue.  First chunk first so
        # the PE can start as soon as the first weight tile lands.
        for (a, w), t in zip(xq_chunks, xtq_t):
            nc.scalar.dma_start(out=t, in_=xtq_r[:, a:a + w, :])
        for (a, w), t in zip(xkv_chunks, xtkv_t):
            nc.scalar.dma_start(out=t, in_=xtkv_r[:, a:a + w, :])
        nc.scalar.dma_start(out=bq_sb, in_=bq[:, :])
        nc.scalar.dma_start(out=bk_sb, in_=bk[:, :])
        nc.scalar.dma_start(out=bv_sb, in_=bv[0:1, :])
        nc.scalar.dma_start(out=b0_sb, in_=b0[0:1, :])

        nc.vector.memset(ones, 1.0)
        nc.vector.memset(mask1, 1.0)
        nc.vector.memset(ones_row, 1.0)
        # mask[p, hk, m=hq*32+ds] = 1.0 iff query s'=ds*16+hq >= key 4p+hk,
        # i.e. 16*ds - 4p + (hq - hk) >= 0, in the permuted column order.
        for hk in range(4):
            for hq in range(16):
                nc.gpsimd.affine_select(
                    out=maskt[:, hk, 32 * hq:32 * (hq + 1)], in_=mask1[:, 0:32],
                    pattern=[[16, 32]], compare_op=GE, fill=0.0,
                    base=hq - hk, channel_multiplier=-4,
                )
        nc.vector.tensor_copy(b0bf, b0_sb)
        nc.vector.tensor_copy(bvbf, bv_sb)

        # Q: four quarter-column passes over WQ, 4 psum banks each -- with
        # the 8-slot psum ring, each pass's banks were drained a full pass
        # ago, so pass boundaries never stall.  First two weight tiles
        # single-cc (fast start), then 2-cc batched.
        wq_chunks = [(0, 1), (1, 1)] + [(2 + 2 * i, 2) for i in range(7)]
        wq_c = wq[:, :].rearrange("(cc p) n -> p cc n", p=128)
        for qp in range(4):
            pq = [pps.tile([128, 512], F32, tag="pj", name=f"pq{qp}_{i}")
                  for i in range(4)]
            for a, w in wq_chunks:
                wt = ws.tile([128, 2, 512], BF16, tag="wq", bufs=12)
                nc.sync.dma_start(out=wt[:, 0:w, :],
                                  in_=wq_c[:, a:a + w, qp * 512:(qp + 1) * 512])
                for sub in range(w):
                    cc = a + sub
                    for ci in range(4):
                        nc.tensor.matmul(pq[ci], wt[:, sub, ci * 128:(ci + 1) * 128],
                                         x_chunk(xtq_t, xq_chunks, cc),
                                         start=(cc == 0), stop=(cc == 15))
            # drains split across vector and scalar so the next psum pass
            # isn't gated on a single engine's drain rate
            for ci in range(4):
                h = qp * 4 + ci
                if ci % 2 == 0:
                    nc.vector.tensor_scalar_add(
                        qt_h[:, :, h, :],
                        pq[ci].rearrange("p (gq ds) -> p gq ds", gq=16),
                        bq_sb[:, h:h + 1])
                else:
                    nc.scalar.activation(
                        qt_h[:, :, h, :],
                        pq[ci].rearrange("p (gq ds) -> p gq ds", gq=16),
                        IDENT, bias=bq_sb[:, h:h + 1])

        # K: one pass, 4 banks, 4-cc batched weight DMAs
        wk_c = wk[:, :].rearrange("(cc p) n -> p cc n", p=128)
        pk = [pps.tile([128, 512], F32, tag="pj", name=f"pk_{i}") for i in range(4)]
        for ccp in range(4):
            wt = ws.tile([128, 4, 512], BF16, tag="wk", bufs=4)
            nc.sync.dma_start(out=wt, in_=wk_c[:, 4 * ccp:4 * ccp + 4, :])
            for sub in range(4):
                cc = 4 * ccp + sub
                for h in range(4):
                    nc.tensor.matmul(pk[h], wt[:, sub, h * 128:(h + 1) * 128],
                                     x_chunk(xtkv_t, xkv_chunks, cc),
                                     start=(cc == 0), stop=(cc == 15))
        for h in range(4):
            if h % 2 == 0:
                nc.vector.tensor_scalar_add(kt[:, h * 512:(h + 1) * 512], pk[h],
                                            bk_sb[:, h:h + 1])
            else:
                nc.scalar.activation(kt[:, h * 512:(h + 1) * 512], pk[h],
                                     IDENT, bias=bk_sb[:, h:h + 1])

        # first two attention blocks' score pipelines, pre-emitted into the
        # V-projection passes below (they only need qt/kt, not vsb)
        def pj_ps(g, qb, m2, h):
            return pps.tile([128, 512], F32, tag="pj", name=f"st_{g}_{qb}_{m2}_{h}")

        pts0, ems0 = make_emitters(0, 0, pj_ps)
        pts1, ems1 = make_emitters(0, 1, pj_ps)
        vpre = {1: ems0, 2: ems1[0:4], 3: ems1[4:8]}

        # V natural [s, (h,dv)]; four (st-pair, nb-pair) passes, 4 psum
        # banks each.  Bias row seeds the accumulators (K=1 matmul), so the
        # drains are plain copies and can split across vector/scalar.
        wv_c = wv[:, :].rearrange("(cc p) n -> p cc n", p=128)
        for stp in range(2):
            for nbp in range(2):
                vp = stp * 2 + nbp
                ems = vpre.get(vp, [])
                psv = [pps.tile([128, 512], F32, tag="pj", name=f"psv{stp}{nbp}_{i}")
                       for i in range(4)]
                for i in range(4):
                    nb = nbp * 2 + i % 2
                    nc.tensor.matmul(psv[i], ones_row,
                                     bvbf[0:1, nb * 512:(nb + 1) * 512],
                                     start=True, stop=False)
                for ccp in range(8):
                    wt = ws.tile([128, 2, 1024], BF16, tag="wv", bufs=6)
                    nc.sync.dma_start(
                        out=wt,
                        in_=wv_c[:, 2 * ccp:2 * ccp + 2, nbp * 1024:(nbp + 1) * 1024])
                    for sub in range(2):
                        cc = 2 * ccp + sub
                        for sti in range(2):
                            st = stp * 2 + sti
                            for nbi in range(2):
                                nc.tensor.matmul(
                                    psv[sti * 2 + nbi],
                                    x_chunk(xtkv_t, xkv_chunks, cc)[
                                        :, st * 128:(st + 1) * 128],
                                    wt[:, sub, nbi * 512:(nbi + 1) * 512],
                                    start=False, stop=(cc == 15))
                    if ems and ccp % 2 == 1:
                        ems[ccp // 2]()
                for sti in range(2):
                    st = stp * 2 + sti
                    for nbi in range(2):
                        nb = nbp * 2 + nbi
                        if nbi == 0:
                            nc.vector.tensor_copy(
                                vsb_t[st][:, nb * 512:(nb + 1) * 512],
                                psv[sti * 2 + nbi])
                        else:
                            nc.scalar.activation(
                                vsb_t[st][:, nb * 512:(nb + 1) * 512],
                                psv[sti * 2 + nbi],
                                mybir.ActivationFunctionType.Copy)

    # W0 stream: issue everything now on the sync queue; the ring guards
    # (bufs=12) park the queue and self-pace the prefetch through the
    # attention phase, where DMA is otherwise idle.
    w0_r = w0[:, :].rearrange("(cj p) n -> p cj n", p=128)   # cj = j*4 + dvc
    w0pool = ctx.enter_context(tc.tile_pool(name="w0s", bufs=1))
    w0tiles = []
    for nb in range(4):
        for cjp in range(32):
            wt0 = w0pool.tile([128, 2, 512], BF16, tag="w0", bufs=16,
                              name=f"w0_{nb}_{cjp}")
            nc.sync.dma_start(
                out=wt0, in_=w0_r[:, 2 * cjp:2 * cjp + 2, nb * 512:(nb + 1) * 512])
            w0tiles.append(wt0)

    # ---------------- attention ----------------
    otpool = ctx.enter_context(tc.tile_pool(name="otpool", bufs=1))
    # [dv-part, g*4+dvc, t] with t ordered (h, qb, ds): the output GEMM's
    # stationary slice for head j is then one contiguous 128-col run.
    ot = otpool.tile([128, 16, 2048], BF16)
    blocks = [(g, qb) for g in range(4) for qb in range(4)]
    with (
        tc.tile_pool(name="rp", bufs=4) as rp,
        tc.tile_pool(name="stps", bufs=3, space="PSUM") as stps,
        tc.tile_pool(name="otps", bufs=4, space="PSUM") as otps,
        tc.tile_pool(name="sumps", bufs=1, space="PSUM") as sumps,
    ):
        def st_ps(g, qb, m2, h):
            return stps.tile([128, 512], F32, tag="st",
                             name=f"st_{g}_{qb}_{m2}_{h}")

        pre_pts = {0: pts0, 1: pts1}
        cur_pts = pts0
        for bi, (g, qb) in enumerate(blocks):
            n = 4 * (qb + 1)
            assert len(cur_pts) == n
            if bi + 1 >= len(blocks):
                next_pts, next_ems = [], []
            elif bi + 1 in pre_pts:
                next_pts, next_ems = pre_pts[bi + 1], []
            else:
                next_pts, next_ems = make_emitters(*blocks[bi + 1], st_ps)
            otp = [otps.tile([128, 512], F32, tag="otp", name=f"otp{p}_{g}_{qb}")
                   for p in range(4)]
            smp = sumps.tile([128, 512], F32, tag="smp", name=f"smp_{g}_{qb}")
            # pass 0: dvc0 + rowsum interleaved
            for i, (m2, h, pt) in enumerate(cur_pts):
                first, last = (i == 0), (i == n - 1)
                nc.tensor.matmul(otp[0],
                                 vsb_t[m2][:, h * 512: h * 512 + 128],
                                 pt, start=first, stop=last)
                nc.tensor.matmul(smp, ones, pt, start=first, stop=last)
            rcb = rp.tile([128, 512], F32, tag="rcb", name=f"rcb_{g}_{qb}")
            nc.vector.reciprocal(rcb, smp)
            # passes 1-3, with the next block's score pipeline interleaved
            total_slots = 3 * n
            slot = 0
            done = 0
            for p in range(1, 4):
                for i, (m2, h, pt) in enumerate(cur_pts):
                    first, last = (i == 0), (i == n - 1)
                    nc.tensor.matmul(
                        otp[p],
                        vsb_t[m2][:, h * 512 + p * 128: h * 512 + (p + 1) * 128],
                        pt, start=first, stop=last)
                    slot += 1
                    # front-loaded: all of the next block's scores are
                    # emitted by mid-pass-2, so exp+mask latency is hidden
                    want = min(len(next_ems),
                               (2 * slot * len(next_ems)) // total_slots)
                    while done < want:
                        next_ems[done]()
                        done += 1
                # drain the pass finished one iteration ago
                nc.vector.tensor_mul(
                    ot[:, 4 * g + p - 1, :].rearrange(
                        "p (h qb ds) -> p h qb ds", h=16, qb=4)[:, :, qb, :],
                    otp[p - 1].rearrange("p (h ds) -> p h ds", h=16),
                    rcb.rearrange("p (h ds) -> p h ds", h=16))
            while done < len(next_ems):
                next_ems[done]()
                done += 1
            nc.vector.tensor_mul(
                ot[:, 4 * g + 3, :].rearrange(
                    "p (h qb ds) -> p h qb ds", h=16, qb=4)[:, :, qb, :],
                otp[3].rearrange("p (h ds) -> p h ds", h=16),
                rcb.rearrange("p (h ds) -> p h ds", h=16))
            cur_pts = next_pts

    # ---------------- output GEMM ----------------
    with (
        tc.tile_pool(name="yp", bufs=4) as yp,
        tc.tile_pool(name="yps", bufs=8, space="PSUM") as yps,
    ):
        ti = 0
        for nb in range(4):
            ypsum = [yps.tile([128, 512], F32, tag="y", name=f"yps{nb}_{g}")
                     for g in range(4)]
            # bias row: y += ones_col^T(K=1) @ b0_row
            for g in range(4):
                nc.tensor.matmul(
                    ypsum[g], ones_row, b0bf[0:1, nb * 512:(nb + 1) * 512],
                    start=True, stop=False)
            for cjp in range(32):
                wt0 = w0tiles[ti]
                ti += 1
                for sub in range(2):
                    cj = 2 * cjp + sub
                    j, dvc = cj // 4, cj % 4
                    for g in range(4):
                        lt = ot[:, 4 * g + dvc, 128 * j:128 * (j + 1)]
                        nc.tensor.matmul(ypsum[g], lt, wt0[:, sub, :],
                                         start=False, stop=(cj == 63))
            for g in range(4):
                yt = yp.tile([128, 512], BF16, tag="y", name=f"yt{nb}_{g}")
                nc.vector.tensor_copy(yt, ypsum[g])
                nc.scalar.dma_start(
                    out=out[g * 128:(g + 1) * 128, nb * 512:(nb + 1) * 512],
                    in_=yt)


def build_graph():
    nc = bacc.Bacc(None, target_bir_lowering=False)
    xtq = nc.declare_dram_parameter("xtq", [D, 512], BF16, isOutput=False)
    xtkv = nc.declare_dram_parameter("xtkv", [D, 512], BF16, isOutput=False)
    wq = nc.declare_dram_parameter("wq", [D, D], BF16, isOutput=False)
    wk = nc.declare_dram_parameter("wk", [D, 512], BF16, isOutput=False)
    wv = nc.declare_dram_parameter("wv", [D, D], BF16, isOutput=False)
    w0 = nc.declare_dram_parameter("w0", [8192, D], BF16, isOutput=False)
    bq = nc.declare_dram_parameter("bq", [128, 16], F32, isOutput=False)
    bk = nc.declare_dram_parameter("bk", [128, 4], F32, isOutput=False)
    bv = nc.declare_dram_parameter("bv", [1, D], F32, isOutput=False)
    b0 = nc.declare_dram_parameter("b0", [1, D], F32, isOutput=False)
    out = nc.declare_dram_parameter("out", [512, D], BF16, isOutput=True)
    from contextlib import ExitStack
    with tile.TileContext(nc) as tc, ExitStack() as ctx:
        _body(tc, ctx, xtq, xtkv, wq, wk, wv, w0, bq, bk, bv, b0, out)
    nc.finalize()
    return nc


_CACHE = {}


def _get_nc():
    if "nc" not in _CACHE:
        _CACHE["nc"] = build_graph()
    return _CACHE["nc"]


def _prep_in_maps(x, WQ, bQ, WK, bK, WV, bV, W0, b0):
    bf16 = mybir.dt.np(BF16)
    x = np.asarray(x, np.float32)
    w0_bf = np.ascontiguousarray(np.asarray(W0, np.float32).astype(bf16))
    wq_bf = np.ascontiguousarray(np.asarray(WQ, np.float32).astype(bf16))
    wk_bf = np.ascontiguousarray(np.asarray(WK, np.float32).astype(bf16))
    wv_bf = np.ascontiguousarray(np.asarray(WV, np.float32).astype(bf16))
    bq_r = np.ascontiguousarray(np.asarray(bQ, np.float32).reshape(16, 128).T)
    bk_r = np.ascontiguousarray(np.asarray(bK, np.float32).reshape(4, 128).T)
    bv_r = np.ascontiguousarray(np.asarray(bV, np.float32).reshape(1, D))
    b0_r = np.ascontiguousarray(np.asarray(b0, np.float32).reshape(1, D))
    in_maps = []
    for core in range(8):
        b, k = core // 4, core % 4
        q_rows = np.concatenate(
            [np.arange((4 * g + k) * 128, (4 * g + k + 1) * 128) for g in range(4)])
        xtq = np.ascontiguousarray(x[b, q_rows, :].T.astype(bf16))
        xtkv = np.ascontiguousarray(x[b, 512 * k:512 * (k + 1), :].T.astype(bf16))
        in_maps.append({
            "xtq": xtq, "xtkv": xtkv, "wq": wq_bf, "wk": wk_bf, "wv": wv_bf,
            "w0": w0_bf, "bq": bq_r, "bk": bk_r, "bv": bv_r, "b0": b0_r,
        })
    return in_maps


def _install_ntff_hook_shim():
    """The image's antenv lacks axon_hooks; provide it so trace=True works."""
    import sys
    import types
    if "antenv.axon_hooks" in sys.modules:
        return
    mod = types.ModuleType("antenv.axon_hooks")
    mod._hook = None

    def set_axon_ntff_profile_hook(h):
        mod._hook = h

    def get_axon_ntff_profile_hook():
        return mod._hook

    mod.set_axon_ntff_profile_hook = set_axon_ntff_profile_hook
    mod.get_axon_ntff_profile_hook = get_axon_ntff_profile_hook
    sys.modules["antenv.axon_hooks"] = mod
    try:
        from trn_agent_boot.trn_boot import _ntff_profile_via_ctypes
        mod._hook = _ntff_profile_via_ctypes("/opt/axon/libaxon_pjrt.so")
    except Exception as e:  # pragma: no cover
        print("ntff shim: hook unavailable:", e)


def run(inputs, trace=False, tmpdir=None, return_res=False):
    """Run on 8 cores; returns (full_output, exec_time_ns_or_None)."""
    if trace:
        _install_ntff_hook_shim()
    in_maps = _prep_in_maps(
        inputs["x"], inputs["WQ"], inputs["bQ"], inputs["WK"], inputs["bK"],
        inputs["WV"], inputs["bV"], inputs["W0"], inputs["b0"])
    res = run_bass_kernel_spmd(_get_nc(), in_maps, core_ids=list(range(8)), trace=trace,
                               tmpdir=tmpdir)
    full = np.zeros((2, 2048, 2048), np.float32)
    for core in range(8):
        b, k = core // 4, core % 4
        co = np.asarray(res.results[core]["out"], dtype=np.float32)
        for g in range(4):
            full[b, (4 * g + k) * 128:(4 * g + k + 1) * 128, :] = co[g * 128:(g + 1) * 128, :]
    if return_res:
        return full, res
    return full, res.exec_time_ns


def kernel(**inputs):
    out, _ = run(inputs, trace=False)
    return out
